# revision 51
# baseline (speedup 1.0000x reference)
"""Trainium2 Bass kernel for nn_CalculateSLayer (GNN message passing).

Computes, for adj (N, N, 2) f32 and s (N, D) f32:
    a     = adj.sum(axis=2)                  # (N, N)
    s_in  = a.T @ s                          # (N, D)
    s_out = a @ s                            # (N, D)
returns (s_in, s_out) — matching the reference's output tuple.

Distribution: adjacency is sharded row-wise across 8 NeuronCores.  Core c
owns rows I_c = [c*512, (c+1)*512).  From its (512, 4096, 2) block it
computes on-device:
  * a partial s_in^T (D, N)    = (s[I_c]).T @ a[I_c]       (contracts i)
  * its exact  s_out^T (D,512) from a[I_c].T               (contracts j)
The host sums the 8 s_in partials and concatenates the s_out blocks.

Per-core dataflow (pipelined under Tile/Bacc; ~47 us HBM roofline):
  DMA : s_own/s_all are HOST-pre-shuffled into [128, tiles, 70] partition-
        major layout so each loads as ONE contiguous descriptor per
        partition (the natural layout needs 280 B descriptors, whose
        per-descriptor overhead starves the adjacency stream for ~6 us —
        SDMA engines round-robin between queues at packet granularity, so
        tiny-descriptor packets hog engine time no matter which HWDGE ring
        carries them).  They ride the ACT HWDGE ring with the identity;
        the 16.8 MB adjacency block streams on the SP ring as 31 x 512 KB
        loads (4 KB/partition descriptors, ~400-420 GB/s) + the last
        i-tile split into 4 x 128 KB so the epilogue starts on a quarter
        tile.  No SWDGE anywhere (its descriptor-ring fetches contend with
        SDMA engines 7/15 and cost ~6 us of input-stream straggle).
  DVE : channel-reduce a_ch[i, j] = raw[i, j, 0] + raw[i, j, 1], casting
        to bf16 on write.
  PE  : all-bf16 with f32 PSUM (walrus rejects mixed 32/16-bit matmul
        inputs; bf16 halves LDWEIGHTS/transpose time vs f32r and halves
        every evacuation):
          s_in matmul  psum_sin(70,512) += s_own[it].T @ a_ch
          transposes   psT[t][j, it*128+i] = a_ch[i, t*128+j]  (bf16 ident)
          s_out matmul psum_out(70,512) += s_all[jt].T @ aT[t]
        s_out matmuls run one chunk behind the transposes so the PE never
        stalls on PSUM evacuation; the final chunk runs lo-half early and
        the hi half chained per 128-column sub-DMA, with evacuations
        alternating ACT/DVE (each copy has ~350 ns fixed overhead).
  DVE : psum_sin -> SBUF staging as bf16 (summed in float64 on the host, so
        the ~1e-3 partial rounding is far under the 2e-2 gate and the
        output traffic that shares SDMA engine time with the input halves).
  DMA : s_in^T chunks 0-6 flush as ONE mid-stream DMA (~1 us of engine
        time instead of 7 separate issues); HBM writes drain at ~190 GB/s
        (write-receipt-bound), so big end-flushes would gate the kernel
        end.  The s_out^T lo half (closed early by its own region stop)
        flushes as soon as its evacuation lands; only the last s_in chunk
        (72 KB) and the 36 KB s_out^T hi half remain in the tail, issued
        from different engines (~0.9 us per DMA_DIRECT2D issue; note
        ACT's HWDGE issue runs ~1.4 us vs SP's ~0.9).

Notes:
  * Measured rel L2 error vs the f32 reference: ~2.8e-3 (gate: 2e-2).
  * The framework preamble (~7 us) and exit teardown (~8.5 us) are fixed
    costs — a 3-instruction kernel measures 15.6 us — so the optimization
    target is purely the DMA window + epilogue between them.
  * Identical binaries measure 63.5-72.7 us across runs (shared-HBM
    environment noise); the stream runs at the SDMA per-packet limit
    (~400-420 GB/s) when the device is quiet.
"""

import ml_dtypes
import numpy as np

import concourse.bass as bass
from concourse import bacc
import concourse.mybir as mybir
import concourse.tile as tile
from concourse import bass_utils

N = 4096          # nodes
D = 70            # embedding dim
NCORES = 8
RB = N // NCORES  # 512 rows per core
P = 128           # partitions
IT = RB // P      # 4 i-tiles per core
WJ = 512          # j-chunk width
JC = N // WJ      # 8 j-chunks
JT = WJ // P      # 4 transpose subtiles per chunk
NJT = N // P      # 32 s_all subtiles

F32 = mybir.dt.float32
F32R = mybir.dt.float32r
BF16 = mybir.dt.bfloat16

# Set by the test harness to capture a profile; the grading path leaves these
# untouched.
TRACE = False
TRACE_KWARGS = {}
LAST_RESULT = None


def _emit(nc: bass.Bass, adj_blk, s_own, s_all, s_inT, s_outT):
    with tile.TileContext(nc) as tc:
        with (
            # one buffer per (chunk, i-tile): no slot reuse, maximal prefetch
            tc.tile_pool(name="raw", bufs=JC * IT + 8) as raw_pool,
            tc.tile_pool(name="work", bufs=1) as work,
            tc.tile_pool(name="singles", bufs=1) as singles,
            tc.tile_pool(name="psT", bufs=1, space="PSUM") as psT_pool,
            tc.tile_pool(name="psSin", bufs=1, space="PSUM") as psSin_pool,
            tc.tile_pool(name="psOut", bufs=1, space="PSUM") as psOut_pool,
        ):
            # (i_tile, partition) view of the raw block
            adj_r = adj_blk.rearrange("(t p) j k -> p t j k", p=P)

            # constants + host-pre-shuffled s tensors on the ACT HWDGE ring:
            # contiguous per-partition bf16 lines, no tiny descriptors
            ident_dram = nc.inline_tensor(
                np.eye(P).astype(ml_dtypes.bfloat16), name="ident_const"
            )
            ident = singles.tile([P, P], BF16)
            nc.scalar.dma_start(out=ident, in_=ident_dram.ap())
            s_own_sb = singles.tile([P, IT, D], BF16)
            nc.scalar.dma_start(out=s_own_sb, in_=s_own)
            s_all_sb = singles.tile([P, NJT, D], BF16)
            nc.scalar.dma_start(out=s_all_sb, in_=s_all)

            # issue every raw load up front: per-(chunk, i-tile) granularity
            # so the first adds start as soon as 512 KB lands; the DMA queue
            # then streams the full 16.8 MB back-to-back at HBM rate.  The
            # very last i-tile is split into 4 x 128-column sub-loads so the
            # epilogue's critical path starts on a quarter tile.
            # the last five full tiles load as halves: their completions
            # stagger ~2x finer, so the tail chunks' adds/transposes chain
            # into the PE's completion-wait gaps instead of bunching
            halved = {(JC - 2, 2), (JC - 2, 3), (JC - 1, 0), (JC - 1, 1),
                      (JC - 1, 2)}
            H2 = WJ // 2
            raws = [[None] * IT for _ in range(JC)]
            last_subs = [None] * JT
            for jc in range(JC):
                for it in range(IT):
                    if jc == JC - 1 and it == IT - 1:
                        for t in range(JT):
                            r = raw_pool.tile([P, P, 2], F32, tag="raw")
                            nc.sync.dma_start(
                                out=r,
                                in_=adj_r[
                                    :,
                                    it,
                                    jc * WJ + t * P : jc * WJ + (t + 1) * P,
                                    :,
                                ],
                            )
                            last_subs[t] = r
                    elif (jc, it) in halved:
                        pair = []
                        for h in range(2):
                            r = raw_pool.tile([P, H2, 2], F32, tag="raw")
                            nc.sync.dma_start(
                                out=r,
                                in_=adj_r[
                                    :,
                                    it,
                                    jc * WJ + h * H2 : jc * WJ + (h + 1) * H2,
                                    :,
                                ],
                            )
                            pair.append(r)
                        raws[jc][it] = pair
                    else:
                        r = raw_pool.tile([P, WJ, 2], F32, tag="raw")
                        nc.sync.dma_start(
                            out=r, in_=adj_r[:, it, jc * WJ : (jc + 1) * WJ, :]
                        )
                        raws[jc][it] = r

            # persistent working tiles
            # the transpose path runs in bf16: the DVE channel-add casts on
            # write, the PE transposes/moves bf16 at full rate, and the
            # evacuations move half the bytes.  s_own/s_all stay f32r (mixed
            # f32r-weights x bf16-moving matmuls are supported; only f32
            # may not be mixed).  Cost: ~1e-3 extra rel error vs a 2e-2 gate.
            a_chs = [
                [
                    work.tile([P, WJ], BF16, name=f"a_ch_{par}_{it}")
                    for it in range(IT)
                ]
                for par in range(2)
            ]
            aTs = [
                [work.tile([P, RB], BF16, name=f"aT_{par}_{t}") for t in range(JT)]
                for par in range(2)
            ]
            # staging split head/tail: chunks 0..6 flush as one mid-stream
            # DMA (per-chunk flushes stole ~1.6 us of SDMA engine time from
            # the engine-bound input stream); the final chunk stages in its
            # own tile so the tail flush reads a contiguous line
            sin_sb_all = work.tile([D, (JC - 1) * WJ], BF16, name="sin_sb_all")
            sin_sb7 = work.tile([D, WJ], BF16, name="sin_sb7")
            psT = [
                psT_pool.tile([P, RB], BF16, name=f"psT_{t}") for t in range(JT)
            ]
            psum_sins = [
                psSin_pool.tile([D, WJ], F32, name=f"psum_sin_{par}")
                for par in range(2)
            ]
            psum_out = psOut_pool.tile([D, RB], F32)

            def emit_sout_mm(jc, t):
                """One s_out^T accumulation for chunk jc, subtile t (aT
                already evacuated; runs one chunk behind so the PE never
                stalls on the PSUM->SBUF copies)."""
                jt = jc * JT + t
                nc.tensor.matmul(
                    psum_out,
                    lhsT=s_all_sb[:, jt, :],
                    rhs=aTs[jc % 2][t],
                    start=(jt == 0),
                    stop=False,
                )

            for jc in range(JC):
                par = jc % 2
                psum_sin = psum_sins[par]
                last = jc == JC - 1
                for it in range(IT):
                    a_ch = a_chs[par][it]
                    if last and it == IT - 1:
                        break
                    raw = raws[jc][it]
                    if (jc, it) in halved:
                        # per-half add + its two transposes, chained on each
                        # half-DMA's completion
                        for h in range(2):
                            nc.vector.tensor_add(
                                out=a_ch[:, h * H2 : (h + 1) * H2],
                                in0=raw[h][:, :, 0],
                                in1=raw[h][:, :, 1],
                            )
                            for t in (2 * h, 2 * h + 1):
                                nc.tensor.transpose(
                                    psT[t][:, it * P : (it + 1) * P],
                                    a_ch[:, t * P : (t + 1) * P],
                                    ident,
                                )
                    else:
                        nc.vector.tensor_add(
                            out=a_ch, in0=raw[:, :, 0], in1=raw[:, :, 1]
                        )
                        # a^T tiles: psT[t][j, it*128 + i] = a[i, t*128 + j]
                        for t in range(JT):
                            nc.tensor.transpose(
                                psT[t][:, it * P : (it + 1) * P],
                                a_ch[:, t * P : (t + 1) * P],
                                ident,
                            )
                    # s_in^T partial: psum_sin[d, j] += sum_i s_own[i, d]*a[i, j]
                    nc.tensor.matmul(
                        psum_sin,
                        lhsT=s_own_sb[:, it, :],
                        rhs=a_ch,
                        start=(it == 0),
                        stop=(it == IT - 1),
                    )
                    if jc > 0:
                        # previous chunk's s_out matmul for subtile `it`: its
                        # aT operand finished copying while this chunk
                        # transposed; one per i-tile position so the four
                        # matmuls fill PE bubbles instead of queueing behind
                        # all of this chunk's transposes
                        emit_sout_mm(jc - 1, it)
                if not last:
                    # evacuate s_in^T chunk (DVE, f32->bf16) and a^T tiles
                    # (ACT)
                    nc.vector.tensor_copy(
                        out=sin_sb_all[:, jc * WJ : (jc + 1) * WJ], in_=psum_sin
                    )
                    for t in range(JT):
                        nc.scalar.copy(out=aTs[par][t], in_=psT[t])
                    if jc == JC - 2:
                        # flush chunks 0..6 as one mid-stream DMA; only the
                        # final chunk's 72 KB is left for the tail (HBM
                        # writes drain at ~190 GB/s, so a 573 KB end-flush
                        # would gate the kernel end).  Issued from the idle
                        # SP engine — its ~1.4 us DMA_DIRECT2D was blocking
                        # the final chunk's aT evacuations on ACT.
                        nc.sync.dma_start(out=s_inT[0], in_=sin_sb_all)

            # ---- epilogue: final chunk (jc = JC-1), i-tiles 0-2 done above.
            # lo half (i-tiles 0-1) evacuates + accumulates as soon as its
            # transposes land; i-tile 2 evacuates per quarter; the last
            # i-tile arrives as 4 x 128-column sub-DMAs, each chaining
            # add -> transpose -> quarter-evac -> hi-half matmul.
            lpar = (JC - 1) % 2
            lo, hi = slice(0, 2 * P), slice(2 * P, RB)
            # chunk JC-2's last s_out matmul (the main loop emitted t=0..2 at
            # the final chunk's i-tile positions, but skipped its last i-tile)
            emit_sout_mm(JC - 2, JT - 1)
            # evacuations alternate ACT/DVE so neither engine's ~450 ns
            # per-copy cost serializes the whole tail
            def evac(t, sl):
                if t % 2 == 0:
                    nc.scalar.copy(out=aTs[lpar][t][:, sl], in_=psT[t][:, sl])
                else:
                    nc.vector.tensor_copy(
                        out=aTs[lpar][t][:, sl], in_=psT[t][:, sl]
                    )

            for t in range(JT):
                evac(t, lo)
            # the lo region's accumulation closes with ITS last matmul (stop
            # is sim/Tile bookkeeping, free on HW), so its s_out^T half
            # evacuates ~2 us before the hi half's chain completes — only
            # the hi evacuation remains on the final critical path
            s_outT_sb = singles.tile([D, RB], BF16)
            for t in range(JT):
                jt = (JC - 1) * JT + t
                nc.tensor.matmul(
                    psum_out[:, lo],
                    lhsT=s_all_sb[:, jt, :],
                    rhs=aTs[lpar][t][:, lo],
                    start=False,
                    stop=(t == JT - 1),
                )
            nc.vector.tensor_copy(out=s_outT_sb[:, lo], in_=psum_out[:, lo])
            # ...and its DMA ships immediately (idle SP engine), leaving only
            # the 36 KB hi half in the final chain
            nc.sync.dma_start(out=s_outT[0], in_=s_outT_sb[:, lo])
            # last i-tile: per-sub chain; each hi half ([it2|it3] columns)
            # evacuates as one copy once its q3 transpose lands — a copy has
            # ~350 ns fixed overhead, so fewer/larger beats quarter-sized
            a_ch3 = a_chs[lpar][IT - 1]
            q3 = slice(3 * P, RB)
            for t in range(JT):
                sub = last_subs[t]
                nc.vector.tensor_add(
                    out=a_ch3[:, t * P : (t + 1) * P],
                    in0=sub[:, :, 0],
                    in1=sub[:, :, 1],
                )
                nc.tensor.transpose(
                    psT[t][:, q3], a_ch3[:, t * P : (t + 1) * P], ident
                )
                evac(t, hi)
                if t == JT - 1:
                    # final s_in matmul fills the PE bubble while ACT
                    # evacuates the last quarter tile
                    nc.tensor.matmul(
                        psum_sins[lpar],
                        lhsT=s_own_sb[:, IT - 1, :],
                        rhs=a_ch3,
                        start=False,
                        stop=True,
                    )
                jt = (JC - 1) * JT + t
                nc.tensor.matmul(
                    psum_out[:, hi],
                    lhsT=s_all_sb[:, jt, :],
                    rhs=aTs[lpar][t][:, hi],
                    start=False,
                    stop=(t == JT - 1),
                )
            # final flushes: each output DMA issues from a different engine
            # (~0.9 us per DMA_DIRECT2D — serializing them on one engine was
            # costing ~2 us); s_out^T evacuates lo on ACT, hi on DVE (bf16:
            # halves issue + flight of the very last transfer), then flushes
            # from the idle SP engine while the whole s_in^T staging tile
            # flushes on ACT
            # final evacuations all on DVE (Tile schedules per-engine order
            # by dependency, so a slow ACT DMA-issue can jump ahead of an
            # ACT evacuation and gate the last output DMA — keep ACT out of
            # the s_out^T critical path entirely: it only issues the sin
            # tail flush); s_out^T flushes from the idle SP engine
            nc.vector.tensor_copy(out=sin_sb7, in_=psum_sins[lpar])
            nc.scalar.dma_start(out=s_inT[1], in_=sin_sb7)
            nc.vector.tensor_copy(out=s_outT_sb[:, hi], in_=psum_out[:, hi])
            nc.sync.dma_start(out=s_outT[1], in_=s_outT_sb[:, hi])


def _build() -> bass.Bass:
    nc = bacc.Bacc("TRN2", num_devices=NCORES)
    adj_blk = nc.dram_tensor("adj_blk", [RB, N, 2], F32, kind="ExternalInput")
    # host-pre-shuffled tile layouts: i = t*128 + p  /  j = t*128 + p
    # (bf16: walrus rejects mixed 32/16-bit matmul inputs, so the whole
    # PE pipeline runs bf16 with f32 PSUM accumulation)
    s_own = nc.dram_tensor("s_own", [P, IT, D], BF16, kind="ExternalInput")
    s_all = nc.dram_tensor("s_all", [P, NJT, D], BF16, kind="ExternalInput")
    s_inT = [
        nc.dram_tensor("s_inT_head", [D, (JC - 1) * WJ], BF16, kind="ExternalOutput"),
        nc.dram_tensor("s_inT_tail", [D, WJ], BF16, kind="ExternalOutput"),
    ]
    s_outT = [
        nc.dram_tensor(f"s_outT_{h}", [D, RB // 2], BF16, kind="ExternalOutput")
        for h in range(2)
    ]
    _emit(
        nc,
        adj_blk.ap(),
        s_own.ap(),
        s_all.ap(),
        [t.ap() for t in s_inT],
        [t.ap() for t in s_outT],
    )
    nc.finalize()
    return nc


_nc_cache = None


def kernel(adj: np.ndarray, s: np.ndarray):
    global _nc_cache, LAST_RESULT
    adj = np.ascontiguousarray(np.asarray(adj, dtype=np.float32))
    s = np.ascontiguousarray(np.asarray(s, dtype=np.float32))
    assert adj.shape == (N, N, 2) and s.shape == (N, D)

    if _nc_cache is None:
        _nc_cache = _build()
    nc = _nc_cache

    # partition-major tile shuffles so every DMA line is contiguous
    s_all_h = np.ascontiguousarray(
        s.reshape(NJT, P, D).transpose(1, 0, 2)
    ).astype(ml_dtypes.bfloat16)  # [p, jt, d], j = jt*128 + p
    in_maps = [
        {
            "adj_blk": np.ascontiguousarray(adj[c * RB : (c + 1) * RB]),
            "s_own": np.ascontiguousarray(
                s[c * RB : (c + 1) * RB].reshape(IT, P, D).transpose(1, 0, 2)
            ).astype(ml_dtypes.bfloat16),
            "s_all": s_all_h,
        }
        for c in range(NCORES)
    ]
    res = bass_utils.run_bass_kernel_spmd(
        nc,
        in_maps,
        core_ids=list(range(NCORES)),
        trace=TRACE,
        **TRACE_KWARGS,
    )
    LAST_RESULT = res

    s_in = (
        np.sum(
            [
                np.concatenate(
                    [
                        r["s_inT_head"].astype(np.float64),
                        r["s_inT_tail"].astype(np.float64),
                    ],
                    axis=1,
                )
                for r in res.results
            ],
            axis=0,
        )
        .astype(np.float32)
        .T
    )
    s_out = np.concatenate(
        [
            np.concatenate([r["s_outT_0"], r["s_outT_1"]], axis=1)
            .astype(np.float32)
            .T
            for r in res.results
        ],
        axis=0,
    )
    return (np.ascontiguousarray(s_in), np.ascontiguousarray(s_out))


# revision 52
# speedup vs baseline: 1.1057x; 1.1057x over previous
"""Trainium2 Bass kernel for nn_CalculateSLayer (GNN message passing).

Computes, for adj (N, N, 2) f32 and s (N, D) f32:
    a     = adj.sum(axis=2)                  # (N, N)
    s_in  = a.T @ s                          # (N, D)
    s_out = a @ s                            # (N, D)
returns (s_in, s_out) — matching the reference's output tuple.

Distribution: adjacency is sharded row-wise across 8 NeuronCores.  Core c
owns rows I_c = [c*512, (c+1)*512).  From its (512, 4096, 2) block it
computes on-device:
  * a partial s_in^T (D, N)    = (s[I_c]).T @ a[I_c]       (contracts i)
  * its exact  s_out^T (D,512) from a[I_c].T               (contracts j)
The host sums the 8 s_in partials and concatenates the s_out blocks.

Per-core dataflow (pipelined under Tile/Bacc; ~47 us HBM roofline):
  DMA : s_own/s_all are HOST-pre-shuffled into [128, tiles, 70] partition-
        major layout so each loads as ONE contiguous descriptor per
        partition (the natural layout needs 280 B descriptors, whose
        per-descriptor overhead starves the adjacency stream for ~6 us —
        SDMA engines round-robin between queues at packet granularity, so
        tiny-descriptor packets hog engine time no matter which HWDGE ring
        carries them).  They ride the ACT HWDGE ring with the identity;
        the 16.8 MB adjacency block streams on the SP ring as 31 x 512 KB
        loads (4 KB/partition descriptors, ~400-420 GB/s) + the last
        i-tile split into 4 x 128 KB so the epilogue starts on a quarter
        tile.  No SWDGE anywhere (its descriptor-ring fetches contend with
        SDMA engines 7/15 and cost ~6 us of input-stream straggle).
  DVE : channel-reduce a_ch[i, j] = raw[i, j, 0] + raw[i, j, 1], casting
        to bf16 on write.
  PE  : all-bf16 with f32 PSUM (walrus rejects mixed 32/16-bit matmul
        inputs; bf16 halves LDWEIGHTS/transpose time vs f32r and halves
        every evacuation):
          s_in matmul  psum_sin(70,512) += s_own[it].T @ a_ch
          transposes   psT[t][j, it*128+i] = a_ch[i, t*128+j]  (bf16 ident)
          s_out matmul psum_out(70,512) += s_all[jt].T @ aT[t]
        s_out matmuls run one chunk behind the transposes so the PE never
        stalls on PSUM evacuation; the final chunk runs lo-half early and
        the hi half chained per 128-column sub-DMA, with evacuations
        alternating ACT/DVE (each copy has ~350 ns fixed overhead).
  DVE : psum_sin -> SBUF staging as bf16 (summed in float64 on the host, so
        the ~1e-3 partial rounding is far under the 2e-2 gate and the
        output traffic that shares SDMA engine time with the input halves).
  DMA : s_in^T chunks 0-6 flush as ONE mid-stream DMA (~1 us of engine
        time instead of 7 separate issues); HBM writes drain at ~190 GB/s
        (write-receipt-bound), so big end-flushes would gate the kernel
        end.  The s_out^T lo half (closed early by its own region stop)
        flushes as soon as its evacuation lands; only the last s_in chunk
        (72 KB) and the 36 KB s_out^T hi half remain in the tail, issued
        from different engines (~0.9 us per DMA_DIRECT2D issue; note
        ACT's HWDGE issue runs ~1.4 us vs SP's ~0.9).

Notes:
  * Measured rel L2 error vs the f32 reference: ~2.8e-3 (gate: 2e-2).
  * The framework preamble (~7 us) and exit teardown (~8.5 us) are fixed
    costs — a 3-instruction kernel measures 15.6 us — so the optimization
    target is purely the DMA window + epilogue between them.
  * Identical binaries measure 63.5-72.7 us across runs (shared-HBM
    environment noise); the stream runs at the SDMA per-packet limit
    (~400-420 GB/s) when the device is quiet.
"""

import ml_dtypes
import numpy as np

import concourse.bass as bass
from concourse import bacc
import concourse.mybir as mybir
import concourse.tile as tile
from concourse import bass_utils

N = 4096          # nodes
D = 70            # embedding dim
NCORES = 8
RB = N // NCORES  # 512 rows per core
P = 128           # partitions
IT = RB // P      # 4 i-tiles per core
WJ = 512          # j-chunk width
JC = N // WJ      # 8 j-chunks
JT = WJ // P      # 4 transpose subtiles per chunk
NJT = N // P      # 32 s_all subtiles

F32 = mybir.dt.float32
F32R = mybir.dt.float32r
BF16 = mybir.dt.bfloat16

# Set by the test harness to capture a profile; the grading path leaves these
# untouched.
TRACE = False
TRACE_KWARGS = {}
LAST_RESULT = None


def _emit(nc: bass.Bass, adj_blk, s_own, s_all, s_inT, s_outT):
    with tile.TileContext(nc) as tc:
        with (
            # one buffer per (chunk, i-tile): no slot reuse, maximal prefetch
            tc.tile_pool(name="raw", bufs=JC * IT + 8) as raw_pool,
            tc.tile_pool(name="work", bufs=1) as work,
            tc.tile_pool(name="singles", bufs=1) as singles,
            tc.tile_pool(name="psT", bufs=1, space="PSUM") as psT_pool,
            tc.tile_pool(name="psSin", bufs=1, space="PSUM") as psSin_pool,
            tc.tile_pool(name="psOut", bufs=1, space="PSUM") as psOut_pool,
        ):
            # (i_tile, partition) view of the raw block
            adj_r = adj_blk.rearrange("(t p) j k -> p t j k", p=P)

            # constants + host-pre-shuffled s tensors on the ACT HWDGE ring:
            # contiguous per-partition bf16 lines, no tiny descriptors
            ident_dram = nc.inline_tensor(
                np.eye(P).astype(ml_dtypes.bfloat16), name="ident_const"
            )
            ident = singles.tile([P, P], BF16)
            nc.scalar.dma_start(out=ident, in_=ident_dram.ap())
            s_own_sb = singles.tile([P, IT, D], BF16)
            nc.scalar.dma_start(out=s_own_sb, in_=s_own)
            s_all_sb = singles.tile([P, NJT, D], BF16)
            nc.scalar.dma_start(out=s_all_sb, in_=s_all)

            # issue every raw load up front: per-(chunk, i-tile) granularity
            # so the first adds start as soon as 512 KB lands; the DMA queue
            # then streams the full 16.8 MB back-to-back at HBM rate.  The
            # very last i-tile is split into 4 x 128-column sub-loads so the
            # epilogue's critical path starts on a quarter tile.
            # the last five full tiles load as halves: their completions
            # stagger ~2x finer, so the tail chunks' adds/transposes chain
            # into the PE's completion-wait gaps instead of bunching
            # (0, 0) is halved too: the stream can't start until the first
            # DMA instruction finishes generating descriptors (~0.6 us for
            # 512), so a half-size first load starts the stream earlier
            halved = {(0, 0), (JC - 2, 2), (JC - 2, 3), (JC - 1, 0),
                      (JC - 1, 1), (JC - 1, 2)}
            H2 = WJ // 2
            raws = [[None] * IT for _ in range(JC)]
            last_subs = [None] * JT
            for jc in range(JC):
                for it in range(IT):
                    if jc == JC - 1 and it == IT - 1:
                        for t in range(JT):
                            r = raw_pool.tile([P, P, 2], F32, tag="raw")
                            nc.sync.dma_start(
                                out=r,
                                in_=adj_r[
                                    :,
                                    it,
                                    jc * WJ + t * P : jc * WJ + (t + 1) * P,
                                    :,
                                ],
                            )
                            last_subs[t] = r
                    elif (jc, it) in halved:
                        pair = []
                        for h in range(2):
                            r = raw_pool.tile([P, H2, 2], F32, tag="raw")
                            nc.sync.dma_start(
                                out=r,
                                in_=adj_r[
                                    :,
                                    it,
                                    jc * WJ + h * H2 : jc * WJ + (h + 1) * H2,
                                    :,
                                ],
                            )
                            pair.append(r)
                        raws[jc][it] = pair
                    else:
                        r = raw_pool.tile([P, WJ, 2], F32, tag="raw")
                        nc.sync.dma_start(
                            out=r, in_=adj_r[:, it, jc * WJ : (jc + 1) * WJ, :]
                        )
                        raws[jc][it] = r

            # persistent working tiles
            # the transpose path runs in bf16: the DVE channel-add casts on
            # write, the PE transposes/moves bf16 at full rate, and the
            # evacuations move half the bytes.  s_own/s_all stay f32r (mixed
            # f32r-weights x bf16-moving matmuls are supported; only f32
            # may not be mixed).  Cost: ~1e-3 extra rel error vs a 2e-2 gate.
            a_chs = [
                [
                    work.tile([P, WJ], BF16, name=f"a_ch_{par}_{it}")
                    for it in range(IT)
                ]
                for par in range(2)
            ]
            aTs = [
                [work.tile([P, RB], BF16, name=f"aT_{par}_{t}") for t in range(JT)]
                for par in range(2)
            ]
            # staging split head/tail: chunks 0..6 flush as one mid-stream
            # DMA (per-chunk flushes stole ~1.6 us of SDMA engine time from
            # the engine-bound input stream); the final chunk stages in its
            # own tile so the tail flush reads a contiguous line
            sin_sb_all = work.tile([D, (JC - 1) * WJ], BF16, name="sin_sb_all")
            sin_sb7 = work.tile([D, WJ], BF16, name="sin_sb7")
            psT = [
                psT_pool.tile([P, RB], BF16, name=f"psT_{t}") for t in range(JT)
            ]
            psum_sins = [
                psSin_pool.tile([D, WJ], F32, name=f"psum_sin_{par}")
                for par in range(2)
            ]
            psum_out = psOut_pool.tile([D, RB], F32)

            def emit_sout_mm(jc, t):
                """One s_out^T accumulation for chunk jc, subtile t (aT
                already evacuated; runs one chunk behind so the PE never
                stalls on the PSUM->SBUF copies)."""
                jt = jc * JT + t
                nc.tensor.matmul(
                    psum_out,
                    lhsT=s_all_sb[:, jt, :],
                    rhs=aTs[jc % 2][t],
                    start=(jt == 0),
                    stop=False,
                )

            for jc in range(JC):
                par = jc % 2
                psum_sin = psum_sins[par]
                last = jc == JC - 1
                for it in range(IT):
                    a_ch = a_chs[par][it]
                    if last and it == IT - 1:
                        break
                    raw = raws[jc][it]
                    if (jc, it) in halved:
                        # per-half add + its two transposes, chained on each
                        # half-DMA's completion
                        for h in range(2):
                            nc.vector.tensor_add(
                                out=a_ch[:, h * H2 : (h + 1) * H2],
                                in0=raw[h][:, :, 0],
                                in1=raw[h][:, :, 1],
                            )
                            for t in (2 * h, 2 * h + 1):
                                nc.tensor.transpose(
                                    psT[t][:, it * P : (it + 1) * P],
                                    a_ch[:, t * P : (t + 1) * P],
                                    ident,
                                )
                    else:
                        nc.vector.tensor_add(
                            out=a_ch, in0=raw[:, :, 0], in1=raw[:, :, 1]
                        )
                        # a^T tiles: psT[t][j, it*128 + i] = a[i, t*128 + j]
                        for t in range(JT):
                            nc.tensor.transpose(
                                psT[t][:, it * P : (it + 1) * P],
                                a_ch[:, t * P : (t + 1) * P],
                                ident,
                            )
                    # s_in^T partial: psum_sin[d, j] += sum_i s_own[i, d]*a[i, j]
                    nc.tensor.matmul(
                        psum_sin,
                        lhsT=s_own_sb[:, it, :],
                        rhs=a_ch,
                        start=(it == 0),
                        stop=(it == IT - 1),
                    )
                    if jc > 0:
                        # previous chunk's s_out matmul for subtile `it`: its
                        # aT operand finished copying while this chunk
                        # transposed; one per i-tile position so the four
                        # matmuls fill PE bubbles instead of queueing behind
                        # all of this chunk's transposes
                        emit_sout_mm(jc - 1, it)
                if not last:
                    # evacuate s_in^T chunk (DVE, f32->bf16) and a^T tiles
                    # (ACT)
                    nc.vector.tensor_copy(
                        out=sin_sb_all[:, jc * WJ : (jc + 1) * WJ], in_=psum_sin
                    )
                    for t in range(JT):
                        nc.scalar.copy(out=aTs[par][t], in_=psT[t])
                    if jc == JC - 2:
                        # flush chunks 0..6 as one mid-stream DMA; only the
                        # final chunk's 72 KB is left for the tail (HBM
                        # writes drain at ~190 GB/s, so a 573 KB end-flush
                        # would gate the kernel end).  Issued from the idle
                        # SP engine — its ~1.4 us DMA_DIRECT2D was blocking
                        # the final chunk's aT evacuations on ACT.
                        nc.sync.dma_start(out=s_inT[0], in_=sin_sb_all)

            # ---- epilogue: final chunk (jc = JC-1), i-tiles 0-2 done above.
            # lo half (i-tiles 0-1) evacuates + accumulates as soon as its
            # transposes land; i-tile 2 evacuates per quarter; the last
            # i-tile arrives as 4 x 128-column sub-DMAs, each chaining
            # add -> transpose -> quarter-evac -> hi-half matmul.
            lpar = (JC - 1) % 2
            lo, hi = slice(0, 2 * P), slice(2 * P, RB)
            # chunk JC-2's last s_out matmul (the main loop emitted t=0..2 at
            # the final chunk's i-tile positions, but skipped its last i-tile)
            emit_sout_mm(JC - 2, JT - 1)
            # evacuations alternate ACT/DVE so neither engine's ~450 ns
            # per-copy cost serializes the whole tail
            def evac(t, sl):
                if t % 2 == 0:
                    nc.scalar.copy(out=aTs[lpar][t][:, sl], in_=psT[t][:, sl])
                else:
                    nc.vector.tensor_copy(
                        out=aTs[lpar][t][:, sl], in_=psT[t][:, sl]
                    )

            for t in range(JT):
                evac(t, lo)
            # the lo region's accumulation closes with ITS last matmul (stop
            # is sim/Tile bookkeeping, free on HW), so its s_out^T half
            # evacuates ~2 us before the hi half's chain completes — only
            # the hi evacuation remains on the final critical path
            s_outT_sb = singles.tile([D, RB], BF16)
            for t in range(JT):
                jt = (JC - 1) * JT + t
                nc.tensor.matmul(
                    psum_out[:, lo],
                    lhsT=s_all_sb[:, jt, :],
                    rhs=aTs[lpar][t][:, lo],
                    start=False,
                    stop=(t == JT - 1),
                )
            nc.vector.tensor_copy(out=s_outT_sb[:, lo], in_=psum_out[:, lo])
            # ...and its DMA ships immediately (idle SP engine), leaving only
            # the 36 KB hi half in the final chain
            nc.sync.dma_start(out=s_outT[0], in_=s_outT_sb[:, lo])
            # last i-tile: per-sub chain; each hi half ([it2|it3] columns)
            # evacuates as one copy once its q3 transpose lands — a copy has
            # ~350 ns fixed overhead, so fewer/larger beats quarter-sized
            a_ch3 = a_chs[lpar][IT - 1]
            q3 = slice(3 * P, RB)
            for t in range(JT):
                sub = last_subs[t]
                nc.vector.tensor_add(
                    out=a_ch3[:, t * P : (t + 1) * P],
                    in0=sub[:, :, 0],
                    in1=sub[:, :, 1],
                )
                nc.tensor.transpose(
                    psT[t][:, q3], a_ch3[:, t * P : (t + 1) * P], ident
                )
                evac(t, hi)
                if t == JT - 1:
                    # final s_in matmul fills the PE bubble while ACT
                    # evacuates the last quarter tile
                    nc.tensor.matmul(
                        psum_sins[lpar],
                        lhsT=s_own_sb[:, IT - 1, :],
                        rhs=a_ch3,
                        start=False,
                        stop=True,
                    )
                jt = (JC - 1) * JT + t
                nc.tensor.matmul(
                    psum_out[:, hi],
                    lhsT=s_all_sb[:, jt, :],
                    rhs=aTs[lpar][t][:, hi],
                    start=False,
                    stop=(t == JT - 1),
                )
            # final flushes: each output DMA issues from a different engine
            # (~0.9 us per DMA_DIRECT2D — serializing them on one engine was
            # costing ~2 us); s_out^T evacuates lo on ACT, hi on DVE (bf16:
            # halves issue + flight of the very last transfer), then flushes
            # from the idle SP engine while the whole s_in^T staging tile
            # flushes on ACT
            # final evacuations all on DVE (Tile schedules per-engine order
            # by dependency, so a slow ACT DMA-issue can jump ahead of an
            # ACT evacuation and gate the last output DMA — keep ACT out of
            # the s_out^T critical path entirely: it only issues the sin
            # tail flush); s_out^T flushes from the idle SP engine
            nc.vector.tensor_copy(out=sin_sb7, in_=psum_sins[lpar])
            nc.scalar.dma_start(out=s_inT[1], in_=sin_sb7)
            nc.vector.tensor_copy(out=s_outT_sb[:, hi], in_=psum_out[:, hi])
            nc.sync.dma_start(out=s_outT[1], in_=s_outT_sb[:, hi])


def _build() -> bass.Bass:
    nc = bacc.Bacc("TRN2", num_devices=NCORES)
    adj_blk = nc.dram_tensor("adj_blk", [RB, N, 2], F32, kind="ExternalInput")
    # host-pre-shuffled tile layouts: i = t*128 + p  /  j = t*128 + p
    # (bf16: walrus rejects mixed 32/16-bit matmul inputs, so the whole
    # PE pipeline runs bf16 with f32 PSUM accumulation)
    s_own = nc.dram_tensor("s_own", [P, IT, D], BF16, kind="ExternalInput")
    s_all = nc.dram_tensor("s_all", [P, NJT, D], BF16, kind="ExternalInput")
    s_inT = [
        nc.dram_tensor("s_inT_head", [D, (JC - 1) * WJ], BF16, kind="ExternalOutput"),
        nc.dram_tensor("s_inT_tail", [D, WJ], BF16, kind="ExternalOutput"),
    ]
    s_outT = [
        nc.dram_tensor(f"s_outT_{h}", [D, RB // 2], BF16, kind="ExternalOutput")
        for h in range(2)
    ]
    _emit(
        nc,
        adj_blk.ap(),
        s_own.ap(),
        s_all.ap(),
        [t.ap() for t in s_inT],
        [t.ap() for t in s_outT],
    )
    nc.finalize()
    return nc


_nc_cache = None


def kernel(adj: np.ndarray, s: np.ndarray):
    global _nc_cache, LAST_RESULT
    adj = np.ascontiguousarray(np.asarray(adj, dtype=np.float32))
    s = np.ascontiguousarray(np.asarray(s, dtype=np.float32))
    assert adj.shape == (N, N, 2) and s.shape == (N, D)

    if _nc_cache is None:
        _nc_cache = _build()
    nc = _nc_cache

    # partition-major tile shuffles so every DMA line is contiguous
    s_all_h = np.ascontiguousarray(
        s.reshape(NJT, P, D).transpose(1, 0, 2)
    ).astype(ml_dtypes.bfloat16)  # [p, jt, d], j = jt*128 + p
    in_maps = [
        {
            "adj_blk": np.ascontiguousarray(adj[c * RB : (c + 1) * RB]),
            "s_own": np.ascontiguousarray(
                s[c * RB : (c + 1) * RB].reshape(IT, P, D).transpose(1, 0, 2)
            ).astype(ml_dtypes.bfloat16),
            "s_all": s_all_h,
        }
        for c in range(NCORES)
    ]
    res = bass_utils.run_bass_kernel_spmd(
        nc,
        in_maps,
        core_ids=list(range(NCORES)),
        trace=TRACE,
        **TRACE_KWARGS,
    )
    LAST_RESULT = res

    s_in = (
        np.sum(
            [
                np.concatenate(
                    [
                        r["s_inT_head"].astype(np.float64),
                        r["s_inT_tail"].astype(np.float64),
                    ],
                    axis=1,
                )
                for r in res.results
            ],
            axis=0,
        )
        .astype(np.float32)
        .T
    )
    s_out = np.concatenate(
        [
            np.concatenate([r["s_outT_0"], r["s_outT_1"]], axis=1)
            .astype(np.float32)
            .T
            for r in res.results
        ],
        axis=0,
    )
    return (np.ascontiguousarray(s_in), np.ascontiguousarray(s_out))
